# revision 23
# baseline (speedup 1.0000x reference)
"""MoE-routed group-norm kernel for Trainium2 (Bass/Tile), 8-core SPMD.

Problem (hardcoded shapes):
  x: [64, 512, 32, 32] f32
  experts_weight/bias: [8, 512], shared_weight/bias: [512]
  router_w: [8, 512], router_b: [8]

  flat = x.mean((2,3)); logits = flat @ router_w.T + router_b
  prob = softmax(logits); top-2 -> coeff = vals / sum(vals)
  fused_w = sum_k coeff_k * experts_weight[idx_k] + shared_weight (bias likewise)
  group-norm over G=32 groups of 16 channels, then y = x_norm * fused_w + fused_b

Strategy: data-parallel over batch, 8 samples per core. Per sample, x[b] is
[512 channels, 1024 spatial] = 4 chunks of [128, 1024] with channels on
partitions. bn_stats gives per-channel mean/var; every cross-partition step
(logits matvec, group-of-16 sums, group->channel broadcast, expert-table
mixing) is a tiny PE matmul against constant masks. Top-2 selection uses the
exp(logit - max) trick: the top-1 exp is exactly 1.0, the softmax denominator
cancels in coeff = vals/sum(vals), so masking with is_lt/is_ge avoids any
index arithmetic. The output pass is a single fused (x * A + B) per chunk with
per-partition scalars A = fused_w * rstd, B = fused_b - mean * A.

Sync-budget notes (walrus codegen allows ONE sync-wait per instruction for
matmul LDWEIGHTS, ACT, and DMA lowerings):
  - PE inputs always have single-engine (DVE) provenance; consts are staged
    through a DVE copy after their DMA.
  - PSUM and ACT-written tiles use static per-sample regions (no slot reuse,
    hence no cross-sample WAW completion waits).
  - Samples are loaded/stored in PAIRS: 4 loads + 4 stores = 8 SWDGE DMAs
    total, so the 8 DMA semaphore lanes are never reused.
  - A 1-element DVE memset on each fresh y tile absorbs the WAR wait against
    the old store, keeping pass2's tensor_scalar at a single wait.
"""

import numpy as np

import concourse.bacc as bacc
import concourse.bass as bass
import concourse.tile as tile
from concourse import mybir
from concourse.bass_utils import run_bass_kernel_spmd

F32 = mybir.dt.float32
ALU = mybir.AluOpType
ACTF = mybir.ActivationFunctionType

P = 128            # SBUF partitions
B, C, HWD = 64, 512, 1024
E, G = 8, 32
EPS = 1e-5
NCORES = 8
BPC = B // NCORES  # samples per core
NCH = C // P       # 4 channel chunks per sample
CPG = C // G       # 16 channels per group
SUB = 512          # bn_stats free-dim max
NSUB = HWD // SUB

# cA layout: [128, 56] = routerT[:, 0:32] | gmask[:, 32:40] | sw[:, 40:44]
#            | sb[:, 44:48] | router_b in row 0 cols 48:56
CA_W = 56
# cB layout: [8, 1152] = bmask[:, 0:128] | ew[:, 128:640] | eb[:, 640:1152]
CB_W = 1152


def build(n_b: int = BPC, pair: int = 2) -> bass.Bass:
    assert n_b % pair == 0
    # Bacc (not plain Bass): its finalize() runs move_matmul_waits_to_ldweights
    # and generate_event_semaphores, which split multi-sem waits to satisfy the
    # one-wait-per-instruction hardware constraint.
    nc = bacc.Bacc()
    x_d = nc.declare_dram_parameter("x", [n_b, C, HWD], F32, isOutput=False)
    ca_d = nc.declare_dram_parameter("ca", [P, CA_W], F32, isOutput=False)
    cb_d = nc.declare_dram_parameter("cb", [E, CB_W], F32, isOutput=False)
    y_d = nc.declare_dram_parameter("y", [n_b, C, HWD], F32, isOutput=True)

    with tile.TileContext(nc) as tc:
        with (
            tc.tile_pool(name="consts", bufs=1) as consts,
            tc.tile_pool(name="xp", bufs=2) as xp,
            tc.tile_pool(name="yp", bufs=2) as yp,
            tc.tile_pool(name="statp", bufs=4) as statp,
            tc.tile_pool(name="tinyp", bufs=3) as tinyp,
            tc.tile_pool(name="ps_static", bufs=1, space="PSUM") as pstat,
        ):
            ca_st = consts.tile([P, CA_W], F32)
            nc.sync.dma_start(out=ca_st, in_=ca_d[:, :])
            cb_st = consts.tile([E, CB_W], F32)
            nc.sync.dma_start(out=cb_st, in_=cb_d[:, :])
            ca = consts.tile([P, CA_W], F32)
            nc.vector.tensor_copy(ca, ca_st)
            cb = consts.tile([E, CB_W], F32)
            nc.vector.tensor_copy(cb, cb_st)
            ones11 = consts.tile([1, 1], F32)
            nc.vector.memset(ones11, 1.0)
            eps8 = consts.tile([E, 1], F32)
            nc.vector.memset(eps8, EPS)

            gmask = ca[:, 32:40]            # [128, 8]: 1 if p//16 == g
            sw = ca[:, 40:44]               # shared_weight chunks [128, 4]
            sb = ca[:, 44:48]               # shared_bias chunks
            rb = ca[0:1, 48:56]             # router bias [1, 8]
            bmask = cb[:, 0:P]              # [8, 128]: 1 if p//16 == g

            # static per-sample regions (see sync-budget notes above)
            ps_sm = pstat.tile([E, 17 * n_b], F32, tag="sm")
            ps_fu = pstat.tile([P, 8 * n_b], F32, tag="fu")
            ps_bc = pstat.tile([P, n_b, NCH, 2], F32, tag="bcx")
            erow_all = consts.tile([1, n_b, E], F32)
            sd_all = consts.tile([E, n_b, NCH], F32)

            for ip in range(n_b // pair):
                x_t2 = xp.tile([P, pair, NCH, HWD], F32)
                nc.gpsimd.dma_start(
                    out=x_t2,
                    in_=x_d[ip * pair : (ip + 1) * pair].rearrange(
                        "b (t p) f -> p b t f", p=P
                    ),
                )
                y_t2 = yp.tile([P, pair, NCH, HWD], F32)
                # absorb the y-slot WAR (old store DMA) into one DVE touch
                nc.vector.memset(y_t2[0:1, 0, 0, 0:1], 0.0)

                for bb in range(pair):
                    b = ip * pair + bb
                    x_t = x_t2[:, bb, :, :]
                    y_t = y_t2[:, bb, :, :]

                    # per-channel mean/var -> mv[:,j,0]=mean, mv[:,j,1]=E[x^2]
                    mv = statp.tile([P, NCH, 2], F32, tag="mv")
                    for j in range(NCH):
                        st6 = statp.tile([P, NSUB, 6], F32, tag="st6")
                        for s in range(NSUB):
                            nc.vector.bn_stats(
                                out=st6[:, s, :],
                                in_=x_t[:, j, s * SUB : (s + 1) * SUB],
                            )
                        nc.vector.bn_aggr(out=mv[:, j, :], in_=st6[:, :, :])
                    msq = tinyp.tile([P, NCH], F32, tag="msq")
                    nc.vector.tensor_tensor(msq, mv[:, :, 0], mv[:, :, 0], ALU.mult)
                    nc.vector.tensor_tensor(mv[:, :, 1], mv[:, :, 1], msq, ALU.add)

                    # small PSUM regions for this sample
                    gs_ps = ps_sm[:, 17 * b : 17 * b + 8]
                    lg_ps = ps_sm[0:1, 17 * b + 8 : 17 * b + 16]
                    ct_ps = ps_sm[:, 17 * b + 16 : 17 * b + 17]
                    # logits[e] = sum_c mean[c] * router_w[e,c] (4-chunk accum)
                    for j in range(NCH):
                        nc.tensor.matmul(
                            lg_ps,
                            mv[:, j, 0:1],
                            ca[:, j * 8 : (j + 1) * 8],
                            start=(j == 0),
                            stop=(j == NCH - 1),
                        )
                    # group sums of per-channel mean / E[x^2]: out[g8, 2j+k]
                    nc.tensor.matmul(gs_ps, gmask, mv[:, :, :])

                    # routing: top-2 coeff over experts without index math
                    lrow = tinyp.tile([1, E], F32, tag="lrow")
                    nc.vector.tensor_tensor(lrow, lg_ps, rb, ALU.add)
                    nmax = tinyp.tile([1, 1], F32, tag="nmax")
                    nc.vector.reduce_max(
                        nmax, lrow, axis=mybir.AxisListType.X, negate=True
                    )
                    erow = erow_all[:, b, :]
                    nc.scalar.activation(erow, lrow, ACTF.Exp, bias=nmax, scale=1.0)
                    qrow = tinyp.tile([1, E], F32, tag="qrow")
                    nc.vector.scalar_tensor_tensor(
                        qrow, erow, 1.0, erow, op0=ALU.is_lt, op1=ALU.mult
                    )
                    m2 = tinyp.tile([1, 1], F32, tag="m2")
                    nc.vector.reduce_max(m2, qrow, axis=mybir.AxisListType.X)
                    gate = tinyp.tile([1, E], F32, tag="gate")
                    nc.vector.scalar_tensor_tensor(
                        gate, erow, m2[0:1, 0:1], erow, op0=ALU.is_ge, op1=ALU.mult
                    )
                    den = tinyp.tile([1, 1], F32, tag="den")
                    nc.vector.tensor_scalar_add(den, m2, 1.0)
                    rden = tinyp.tile([1, 1], F32, tag="rden")
                    nc.vector.reciprocal(rden, den)
                    crow = tinyp.tile([1, E], F32, tag="crow")
                    nc.vector.tensor_scalar_mul(crow, gate, rden[0:1, 0:1])
                    # transpose coeff [1,8] -> [8,1] via K=1 matmul
                    nc.tensor.matmul(ct_ps, crow, ones11)
                    cT = tinyp.tile([E, 1], F32, tag="cT")
                    nc.vector.tensor_copy(cT, ct_ps)

                    # group mean / rstd in [8,4,2] (partition=group-in-chunk)
                    gsj = gs_ps.rearrange("g (j k) -> g j k", k=2)
                    mr = statp.tile([E, NCH, 2], F32, tag="mr")
                    nc.vector.tensor_scalar_mul(mr[:, :, 0], gsj[:, :, 0], 1.0 / CPG)
                    ex2 = tinyp.tile([E, NCH], F32, tag="ex2")
                    nc.vector.tensor_scalar_mul(ex2, gsj[:, :, 1], 1.0 / CPG)
                    mg2 = tinyp.tile([E, NCH], F32, tag="mg2")
                    nc.vector.tensor_tensor(mg2, mr[:, :, 0], mr[:, :, 0], ALU.mult)
                    var = tinyp.tile([E, NCH], F32, tag="var")
                    nc.vector.tensor_tensor(var, ex2, mg2, ALU.subtract)
                    sd = sd_all[:, b, :]
                    nc.scalar.activation(sd, var, ACTF.Sqrt, bias=eps8, scale=1.0)
                    nc.vector.reciprocal(mr[:, :, 1], sd)

                    # bc[p, 2j+k] = stat k of group (8j + p//16)
                    bc = ps_bc[:, b, :, :]
                    nc.tensor.matmul(bc, bmask, mr[:, :, :])
                    # fused expert params: fu[:,j]=w chunk j, fu[:,4+j]=b chunk
                    fu = ps_fu[:, 8 * b : 8 * b + 8]
                    for j in range(NCH):
                        nc.tensor.matmul(
                            fu[:, j : j + 1], cb[:, P + j * P : P + (j + 1) * P], cT
                        )
                        nc.tensor.matmul(
                            fu[:, NCH + j : NCH + j + 1],
                            cb[:, 640 + j * P : 640 + (j + 1) * P],
                            cT,
                        )

                    # A = (fused_w + shared_w) * rstd
                    # B = (fused_b + shared_b) - mean * A
                    t1 = tinyp.tile([P, NCH], F32, tag="t1")
                    nc.vector.tensor_tensor(t1, fu[:, 0:NCH], sw, ALU.add)
                    At = tinyp.tile([P, NCH], F32, tag="At")
                    nc.vector.tensor_tensor(At, t1, bc[:, :, 1], ALU.mult)
                    t2 = tinyp.tile([P, NCH], F32, tag="t2")
                    nc.vector.tensor_tensor(t2, fu[:, NCH : 2 * NCH], sb, ALU.add)
                    t3 = tinyp.tile([P, NCH], F32, tag="t3")
                    nc.vector.tensor_tensor(t3, bc[:, :, 0], At, ALU.mult)
                    Bt = tinyp.tile([P, NCH], F32, tag="Bt")
                    nc.vector.tensor_tensor(Bt, t2, t3, ALU.subtract)

                    for j in range(NCH):
                        nc.vector.tensor_scalar(
                            y_t[:, j, :],
                            x_t[:, j, :],
                            At[:, j : j + 1],
                            Bt[:, j : j + 1],
                            op0=ALU.mult,
                            op1=ALU.add,
                        )

                nc.gpsimd.dma_start(
                    out=y_d[ip * pair : (ip + 1) * pair].rearrange(
                        "b (t p) f -> p b t f", p=P
                    ),
                    in_=y_t2,
                )
    nc.finalize()
    return nc


def pack_consts(
    experts_weight, experts_bias, shared_weight, shared_bias, router_w, router_b
):
    ca = np.zeros((P, CA_W), np.float32)
    # routerT[p, 8j+e] = router_w[e, 128j+p]
    ca[:, 0:32] = (
        np.ascontiguousarray(router_w.T)
        .reshape(NCH, P, E)
        .transpose(1, 0, 2)
        .reshape(P, 32)
    )
    pidx = np.arange(P)
    ca[:, 32:40] = (pidx[:, None] // CPG == np.arange(8)[None, :]).astype(np.float32)
    ca[:, 40:44] = shared_weight.reshape(NCH, P).T
    ca[:, 44:48] = shared_bias.reshape(NCH, P).T
    ca[0, 48:56] = router_b
    cb = np.zeros((E, CB_W), np.float32)
    cb[:, 0:P] = (np.arange(E)[:, None] == pidx[None, :] // CPG).astype(np.float32)
    cb[:, P : P + C] = experts_weight
    cb[:, P + C : P + 2 * C] = experts_bias
    return ca, cb


_NC_CACHE: dict[int, bass.Bass] = {}


def _get_nc(n_b: int) -> bass.Bass:
    if n_b not in _NC_CACHE:
        _NC_CACHE[n_b] = build(n_b)
    return _NC_CACHE[n_b]


def run(
    x,
    experts_weight,
    experts_bias,
    shared_weight,
    shared_bias,
    router_w,
    router_b,
    trace: bool = False,
    tmpdir=None,
):
    x = np.ascontiguousarray(np.asarray(x, np.float32)).reshape(B, C, HWD)
    ca, cb = pack_consts(
        np.asarray(experts_weight, np.float32),
        np.asarray(experts_bias, np.float32),
        np.asarray(shared_weight, np.float32),
        np.asarray(shared_bias, np.float32),
        np.asarray(router_w, np.float32),
        np.asarray(router_b, np.float32),
    )
    nc = _get_nc(BPC)
    in_maps = [
        {"x": x[i * BPC : (i + 1) * BPC], "ca": ca, "cb": cb} for i in range(NCORES)
    ]
    res = run_bass_kernel_spmd(
        nc, in_maps, list(range(NCORES)), trace=trace, tmpdir=tmpdir
    )
    y = np.concatenate([res.results[i]["y"] for i in range(NCORES)], axis=0)
    return y.reshape(B, C, 32, 32), res


def kernel(**inputs) -> np.ndarray:
    y, _ = run(**inputs)
    return y
